# revision 1
# baseline (speedup 1.0000x reference)
"""AttentivePoolingNetwork Trainium2 kernel.

B=256 batch sharded 32/core across 8 NeuronCores. Per core:
  Q = cnn_encode(question)   [C=400(pad512), 32*40]   (bf16 matmuls, f32 psum)
  A = cnn_encode(answer)     [C=400(pad512), 32*400]  (kept in SBUF, bf16)
  P = U^T Q                  [C, 32*40]
  Gpre_b = P_b^T A_b         [40, 400] per batch item (tanh deferred)
  gq[b,m] = max_l Gpre, ga[b,l] = max_m Gpre (tanh applied after max)
  Softmax over the global batch dim via AllReduce(add) of local exp-sums.
  The exp(tanh(ga)) rows are broadcast + multiplied into A *during* phase B
  (Y = A .* exp, in place), so post-collective work is only the 1/S scale
  and a fused free-dim reduce per c-tile:
  rQ = Q w_q, rA = A w_a, out = cos(rQ, rA)
"""

import numpy as np
import ml_dtypes

import concourse.bass as bass
import concourse.tile as tile
from concourse import bacc, mybir
import concourse.bass_isa as bass_isa
from concourse.bass_utils import run_bass_kernel_spmd

F32 = mybir.dt.float32
BF16 = mybir.dt.bfloat16
FP8 = mybir.dt.float8e4
DR = mybir.MatmulPerfMode.DoubleRow
AF = mybir.ActivationFunctionType
OP = mybir.AluOpType

N_CORES = 8
B, M, L, E, C = 256, 40, 400, 300, 400
BS = B // N_CORES          # 32 batch per core
EP = 384                   # E padded to 3*128
CP = 512                   # C padded to 4*128
NT = CP // 128             # 4 c/d tiles
MT = BS * M                # 1280
LT = BS * L                # 12800
CHUNK = 8                  # batch chunk for Q/P matmuls (free dim 320)
NCH = BS // CHUNK          # 4
GRP = 4                    # batch group for A-encode psum rotation
NG = BS // GRP             # 8 groups

_CACHE = {}


def _build():
    nc = bacc.Bacc("TRN2", target_bir_lowering=False)

    xq_d = nc.dram_tensor("xq", [8, 128, MT], BF16, kind="ExternalInput")
    xa_d = nc.dram_tensor("xa", [BS, 8, 128, 400], BF16, kind="ExternalInput")
    wq_d = nc.dram_tensor("wqt", [8, 128, CP], BF16, kind="ExternalInput")
    wa_d = nc.dram_tensor("wat", [8, 128, CP], BF16, kind="ExternalInput")
    u_d = nc.dram_tensor("ut", [NT, 128, CP], BF16, kind="ExternalInput")
    bq_d = nc.dram_tensor("bq", [128, NT], F32, kind="ExternalInput")
    ba_d = nc.dram_tensor("ba", [128, NT], F32, kind="ExternalInput")
    id_d = nc.dram_tensor("ident", [128, 128], F32, kind="ExternalInput")
    on_d = nc.dram_tensor("ones", [1, 128], BF16, kind="ExternalInput")
    oc_d = nc.dram_tensor("ones_col", [128, 1], BF16, kind="ExternalInput")
    out_d = nc.dram_tensor("out", [32], F32, kind="ExternalOutput")

    with tile.TileContext(nc) as tc:
        with tc.tile_pool(name="const", bufs=1) as cp, \
             tc.tile_pool(name="dram", bufs=1, space="DRAM") as dp:
            # ---- persistent SBUF tensors ----
            wa_sb = cp.tile([128, 8 * CP], BF16, tag="wa_sb", name="wa_sb")
            bq_sb = cp.tile([128, NT], F32, tag="bq_sb", name="bq_sb")
            ba_sb = cp.tile([128, NT], F32, tag="ba_sb", name="ba_sb")
            id_sb = cp.tile([128, 128], F32, tag="id_sb", name="id_sb")
            on_sb = cp.tile([1, 128], BF16, tag="on_sb", name="on_sb")
            oc_sb = cp.tile([128, 1], BF16, tag="oc_sb", name="oc_sb")
            q_sb = [cp.tile([128, MT], BF16, tag=f"q_sb{t}", name=f"q_sb{t}") for t in range(NT)]
            a_sb = [cp.tile([128, LT], BF16, tag=f"a_sb{t}", name=f"a_sb{t}") for t in range(NT)]
            gq_all = cp.tile([40, BS], F32, tag="gq_all", name="gq_all")
            s_acc = cp.tile([1, 400], F32, tag="s_acc", name="s_acc")
            sq_acc = cp.tile([40, 1], F32, tag="sq_acc", name="sq_acc")
            rq_t = [cp.tile([128, BS], F32, tag=f"rq{t}", name=f"rq{t}") for t in range(NT)]
            ra_t = [cp.tile([128, BS], F32, tag=f"ra{t}", name=f"ra{t}") for t in range(NT)]

            ccin = dp.tile([1, 440], F32, tag="ccin", name="ccin")
            ccout = dp.tile([1, 440], F32, tag="ccout", name="ccout")
            ccwi = dp.tile([1, 8], F32, tag="ccwi", name="ccwi")
            ccwo = dp.tile([1, 8], F32, tag="ccwo", name="ccwo")

            # ---- load constants/inputs (A-phase gates first) ----
            _pab_cm = tc.tile_pool(name="pab", bufs=1)
            pab_pool = _pab_cm.__enter__()
            p_sb = [pab_pool.tile([128, MT], BF16, tag=f"p_sb{t}", name=f"p_sb{t}") for t in range(NT)]
            _pa_cm = tc.tile_pool(name="pa", bufs=1)
            pa_pool = _pa_cm.__enter__()
            xq_all = pa_pool.tile([128, 8 * MT], BF16, tag="xq_all", name="xq_all")
            wq_sb = pa_pool.tile([128, 8 * CP], BF16, tag="wq_sb", name="wq_sb")
            u_sb = pa_pool.tile([128, NT * CP], BF16, tag="u_sb", name="u_sb")
            # per-k-tile loads: the first Q matmul only waits for slice 0
            for k in range(8):
                nc.sync.dma_start(wq_sb[:, k * CP:(k + 1) * CP], wq_d[k])
                nc.sync.dma_start(xq_all[:, k * MT:(k + 1) * MT], xq_d[k])
            nc.sync.dma_start(bq_sb[:], bq_d[:])
            nc.scalar.dma_start(
                u_sb[:].rearrange("p (k d) -> p k d", k=NT),
                u_d[:].rearrange("k p d -> p k d"))
            nc.scalar.dma_start(
                wa_sb[:].rearrange("p (k c) -> p k c", k=8),
                wa_d[:].rearrange("k p c -> p k c"))
            nc.scalar.dma_start(ba_sb[:], ba_d[:])
            nc.sync.dma_start(id_sb[:], id_d[:])
            nc.sync.dma_start(on_sb[:], on_d[:])
            nc.sync.dma_start(oc_sb[:], oc_d[:])

            # warm the ACT LUT tables used later (table swap costs ~10us
            # if it lands on the critical path)
            warm = cp.tile([1, 32], F32, tag="warm", name="warm")
            nc.vector.memset(warm[:, :], 0.25)
            nc.scalar.activation(warm[:, :], warm[:, :], AF.Tanh)
            nc.scalar.activation(warm[:, :], warm[:, :], AF.Exp)
            nc.scalar.activation(warm[:, :], warm[:, :], AF.Sqrt)

            # dummy collective: pre-pays the CC mesh wakeup and absorbs the
            # cross-core launch skew long before the real AllReduce
            wsync = cp.tile([1, 8], F32, tag="wsync", name="wsync")
            nc.vector.memset(wsync[:, :], 1.0)
            nc.gpsimd.dma_start(ccwi[0:1, :], wsync[:, :])
            nc.gpsimd.collective_compute(
                "AllReduce", OP.add,
                replica_groups=[list(range(N_CORES))],
                ins=[ccwi[:].opt()], outs=[ccwo[:].opt()])

            # ---- Phase A: Q encode + P = U^T Q ----
            with tc.tile_pool(name="qpsum", bufs=8, space="PSUM") as qp:
                for t in range(NT):
                    ps = [qp.tile([128, CHUNK * M], F32, tag="qps", name="qps") for _ in range(NCH)]
                    for k in range(8):
                        lhsT = wq_sb[:, k * CP + t * 128:k * CP + (t + 1) * 128]
                        for s in range(NCH):
                            rhs = xq_all[:, k * MT + s * CHUNK * M:
                                         k * MT + (s + 1) * CHUNK * M]
                            nc.tensor.matmul(ps[s][:, :], lhsT, rhs,
                                             start=(k == 0), stop=(k == 7))
                    for s in range(NCH):
                        nc.vector.tensor_add(
                            q_sb[t][:, s * CHUNK * M:(s + 1) * CHUNK * M],
                            ps[s][:, :],
                            bq_sb[:, t:t + 1].broadcast_to((128, CHUNK * M)))

            with tc.tile_pool(name="ppsum", bufs=8, space="PSUM") as pp:
                for t in range(NT):
                    for s in range(NCH):
                        ps = pp.tile([128, CHUNK * M], F32, tag="pps", name="pps")
                        for kk in range(NT):
                            lhsT = u_sb[:, kk * CP + t * 128:kk * CP + (t + 1) * 128]
                            rhs = q_sb[kk][:, s * CHUNK * M:(s + 1) * CHUNK * M]
                            nc.tensor.matmul(ps[:, :], lhsT, rhs,
                                             start=(kk == 0), stop=(kk == NT - 1))
                        nc.vector.tensor_copy(
                            p_sb[t][:, s * CHUNK * M:(s + 1) * CHUNK * M], ps[:, :])

            _pa_cm.__exit__(None, None, None)

            # ---- Phase B: A encode + G + pooled maxes + exp + Y = A.*exp ----
            nc.vector.memset(s_acc[:, :], 0.0)
            nc.vector.memset(sq_acc[:, :], 0.0)
            with tc.tile_pool(name="xa_pool", bufs=6) as xap, \
                 tc.tile_pool(name="apsum", bufs=5, space="PSUM") as ap, \
                 tc.tile_pool(name="gpsum", bufs=2, space="PSUM") as gp, \
                 tc.tile_pool(name="ebpsum", bufs=1, space="PSUM") as ebp, \
                 tc.tile_pool(name="ebc", bufs=2) as ebcp, \
                 tc.tile_pool(name="rows", bufs=3) as rowp, \
                 tc.tile_pool(name="tree", bufs=2) as trp:

                e_grp = {}

                def do_g(bb):
                    g = gp.tile([64, 400], F32, tag="gps", name="gps")
                    for kk in range(NT):
                        nc.tensor.matmul(
                            g[0:40, :],
                            p_sb[kk][:, bb * M:(bb + 1) * M],
                            a_sb[kk][:, bb * L:(bb + 1) * L],
                            start=(kk == 0), stop=(kk == NT - 1))
                    nc.vector.reduce_max(gq_all[0:40, bb:bb + 1], g[0:40, :],
                                         axis=mybir.AxisListType.X, op=OP.max)
                    # incremental gq-side softmax sum (keeps the collective
                    # run-up to just the ccin DMAs)
                    e1q = rowp.tile([40, 1], F32, tag="e1q", name="e1q")
                    nc.scalar.activation(e1q[:, :], gq_all[0:40, bb:bb + 1],
                                         AF.Tanh)
                    nc.scalar.activation(e1q[:, :], e1q[:, :], AF.Exp)
                    nc.vector.tensor_add(sq_acc[:, :], sq_acc[:, :], e1q[:, :])
                    g_s = trp.tile([40, 400], F32, tag="g_s", name="g_s")
                    nc.scalar.activation(g_s[:, :], g[0:40, :], AF.Copy)
                    g_r = trp.tile([40, 400], F32, tag="g_r", name="g_r")
                    nc.gpsimd.partition_all_reduce(
                        g_r[:, :], g_s[:, :], channels=40,
                        reduce_op=bass_isa.ReduceOp.max)
                    # exp(tanh(max_m G)) row for batch bb
                    t1 = rowp.tile([1, 400], F32, tag="t1", name="t1")
                    nc.scalar.activation(t1[:, :], g_r[0:1, :], AF.Tanh)
                    e1 = rowp.tile([1, 400], F32, tag="e1", name="e1")
                    nc.scalar.activation(e1[:, :], t1[:, :], AF.Exp)
                    # local softmax-sum accumulator (keeps the collective
                    # run-up off the critical path)
                    nc.vector.tensor_add(s_acc[:, :], s_acc[:, :], e1[:, :])
                    # broadcast exp row to 128 partitions for the Y multiply
                    e1b = rowp.tile([1, 400], BF16, tag="e1b", name="e1b")
                    nc.vector.tensor_copy(e1b[:, :], e1[:, :])
                    eps = ebp.tile([128, 400], F32, tag="eps", name="eps")
                    nc.tensor.matmul(eps[:, :], on_sb[0:1, :], e1b[:, :],
                                     start=True, stop=True)
                    eg = e_grp[bb // GRP]
                    nc.scalar.activation(eg[:, (bb % GRP) * 400:(bb % GRP + 1) * 400],
                                         eps[:, :], AF.Copy)

                def tail(grp):
                    # Y = A .* exp-broadcast, in place over this group's slice
                    eg = e_grp.pop(grp)
                    sl = slice(grp * GRP * L, (grp + 1) * GRP * L)
                    for t in range(NT):
                        nc.vector.tensor_tensor(a_sb[t][:, sl], a_sb[t][:, sl],
                                                eg[:, :], op=OP.mult)

                for grp in range(NG):
                    bs0 = grp * GRP
                    e_grp[grp] = ebcp.tile([128, GRP * 400], BF16, tag="e_g", name="e_g")
                    xts = []
                    for bb in range(bs0, bs0 + GRP):
                        xt = xap.tile([128, 8 * 400], BF16, tag="xa_t", name="xa_t")
                        nc.scalar.dma_start(
                            xt[:].rearrange("p (k l) -> p k l", k=8),
                            xa_d[bb].rearrange("k p l -> p k l"))
                        xts.append(xt)
                    for t in range(NT):
                        ps = [ap.tile([128, 400], F32, tag="aps", name="aps") for _ in range(GRP)]
                        for k in range(8):
                            lhsT = wa_sb[:, k * CP + t * 128:k * CP + (t + 1) * 128]
                            for x in range(GRP):
                                rhs = xts[x][:, k * 400:(k + 1) * 400]
                                nc.tensor.matmul(ps[x][:, :], lhsT, rhs,
                                                 start=(k == 0), stop=(k == 7))
                        for x in range(GRP):
                            bb = bs0 + x
                            nc.vector.tensor_add(
                                a_sb[t][:, bb * L:(bb + 1) * L], ps[x][:, :],
                                ba_sb[:, t:t + 1].broadcast_to((128, 400)))
                    if grp > 0:
                        for bb in range(bs0 - GRP, bs0):
                            do_g(bb)
                        tail(grp - 1)
                    if grp == NG - 2:
                        # re-warm the Sqrt LUT so phase C's norm doesn't pay
                        # a demand table load on the critical path
                        nc.scalar.activation(warm[:, :], warm[:, :], AF.Sqrt)
                for bb in range(BS - GRP, BS):
                    do_g(bb)
                tail(NG - 1)

            _pab_cm.__exit__(None, None, None)

            # ---- Phase C: batch softmax (AllReduce) + pooled sums + cosine ----
            with tc.tile_pool(name="phc", bufs=1) as pc, \
                 tc.tile_pool(name="cpsum", bufs=2, space="PSUM") as cps, \
                 tc.tile_pool(name="cpsum1", bufs=2, space="PSUM") as cp1:
                # local softmax sums -> collective, ASAP after last do_g
                nc.gpsimd.dma_start(ccin[0:1, 0:40], sq_acc[:, :])
                nc.gpsimd.dma_start(ccin[0:1, 40:440], s_acc[:, :])
                nc.gpsimd.collective_compute(
                    "AllReduce", OP.add,
                    replica_groups=[list(range(N_CORES))],
                    ins=[ccin[:].opt()], outs=[ccout[:].opt()])

                # --- work hidden under the collective: Yq = Q .* exp_q ---
                tq = pc.tile([40, BS], F32, tag="tq", name="tq")
                nc.scalar.activation(tq[:, :], gq_all[:, :], AF.Tanh)
                e_q = pc.tile([40, BS], F32, tag="e_q", name="e_q")
                nc.scalar.activation(e_q[:, :], tq[:, :], AF.Exp)
                eqt_ps = cp1.tile([BS, 40], F32, tag="c1", name="eqt_ps")
                nc.tensor.transpose(eqt_ps[:, :], e_q[:, :], id_sb[0:40, 0:40])
                eqt = pc.tile([BS, 40], BF16, tag="eqt", name="eqt")
                nc.vector.tensor_copy(eqt[:, :], eqt_ps[:, :])
                eq_fl = pc.tile([1, MT], BF16, tag="eq_fl", name="eq_fl")
                nc.gpsimd.dma_start(eq_fl[0:1, :], eqt[:, :])
                eq_bc = pc.tile([128, MT], BF16, tag="eq_bc", name="eq_bc")
                for ch in range(0, MT, 512):
                    ce = min(ch + 512, MT)
                    wqb = cps.tile([128, 512], F32, tag="wqb", name="wqb")
                    nc.tensor.matmul(wqb[:, 0:ce - ch], on_sb[:, :],
                                     eq_fl[0:1, ch:ce], start=True, stop=True)
                    nc.scalar.activation(eq_bc[:, ch:ce], wqb[:, 0:ce - ch], AF.Copy)
                for t in range(NT):
                    nc.vector.tensor_tensor(q_sb[t][:, :], q_sb[t][:, :],
                                            eq_bc[:, :], op=OP.mult)

                # --- post-collective: u = 1/S, scale + fused reduces ---
                ss = pc.tile([1, 440], F32, tag="ss", name="ss")
                nc.gpsimd.dma_start(ss[:, :], ccout[0:1, :])
                ur = pc.tile([1, 440], F32, tag="ur", name="ur")
                nc.vector.reciprocal_approx_fast(ur[:, :], ss[:, :])
                ub = pc.tile([1, 440], BF16, tag="ub", name="ub")
                nc.vector.tensor_copy(ub[:, :], ur[:, :])
                uaps = cp1.tile([128, 400], F32, tag="c1", name="uaps")
                nc.tensor.matmul(uaps[:, :], on_sb[:, :], ub[0:1, 40:440],
                                 start=True, stop=True)
                ua_bc = pc.tile([128, 400], BF16, tag="ua_bc", name="ua_bc")
                nc.scalar.activation(ua_bc[:, :], uaps[:, :], AF.Copy)
                uqps = cp1.tile([128, 40], F32, tag="c1q", name="uqps")
                nc.tensor.matmul(uqps[:, :], on_sb[:, :], ub[0:1, 0:40],
                                 start=True, stop=True)
                uq_bc = pc.tile([128, 40], BF16, tag="uq_bc", name="uq_bc")
                nc.scalar.activation(uq_bc[:, :], uqps[:, :], AF.Copy)

                ua_v = ua_bc[:].rearrange("p (o l) -> p o l", o=1).broadcast_to((128, BS, 400))
                uq_v = uq_bc[:].rearrange("p (o m) -> p o m", o=1).broadcast_to((128, BS, 40))

                def tree_sum(av, out, n):
                    # pairwise in-place halving keeps every DVE op all-bf16
                    # (2x rate); a strided X-reduce would run at 1x.
                    while n > 25 and n % 2 == 0:
                        h = n // 2
                        nc.vector.tensor_tensor(av[:, :, 0:h], av[:, :, 0:h],
                                                av[:, :, h:n], op=OP.add)
                        n = h
                    nc.vector.reduce_sum(out, av[:, :, 0:n],
                                         axis=mybir.AxisListType.X, op=OP.add)

                scr = pc.tile([128, 400], BF16, tag="scr", name="scr")
                for t in range(NT):
                    av = a_sb[t][:].rearrange("p (b l) -> p b l", b=BS)
                    nc.vector.tensor_tensor(av, av, ua_v, op=OP.mult)
                    if t == 0:
                        # the otherwise-idle scalar engine takes one c-tile's
                        # reduction (runs parallel to the DVE trees)
                        for bb in range(BS):
                            nc.scalar.activation(
                                scr[:, :], av[:, bb, :], AF.Copy,
                                accum_out=ra_t[t][:, bb:bb + 1])
                    else:
                        tree_sum(av, ra_t[t][:, :], 400)
                    qv = q_sb[t][:].rearrange("p (b m) -> p b m", b=BS)
                    nc.vector.tensor_tensor(qv, qv, uq_v, op=OP.mult)
                    tree_sum(qv, rq_t[t][:, :], 40)

                # cosine similarity: reduce over c = 4 tiles x 128 partitions
                # via accumulating PE transposes: psum [BS,128] = sum_t P_t^T,
                # then a free-dim reduce gives the per-b column.
                def psum_all(tiles, tag):
                    tps = cps.tile([BS, 128], F32, tag="cts", name=f"{tag}tp")
                    for t in range(NT):
                        nc.tensor.matmul(tps[:, :], tiles[t][:, :], id_sb[:, :],
                                         is_transpose=True,
                                         start=(t == 0), stop=(t == NT - 1))
                    col = pc.tile([32, 1], F32, tag=f"{tag}c", name=f"{tag}c")
                    nc.vector.reduce_sum(col[:, :], tps[:, :],
                                         axis=mybir.AxisListType.X, op=OP.add)
                    return col

                pr = [pc.tile([128, BS], F32, tag=f"pr{t}", name=f"pr{t}") for t in range(NT)]
                pq = [pc.tile([128, BS], F32, tag=f"pq{t}", name=f"pq{t}") for t in range(NT)]
                pa = [pc.tile([128, BS], F32, tag=f"pa{t}", name=f"pa{t}") for t in range(NT)]
                for t in range(NT):
                    nc.vector.tensor_mul(pr[t][:, :], rq_t[t][:, :], ra_t[t][:, :])
                    nc.vector.tensor_mul(pq[t][:, :], rq_t[t][:, :], rq_t[t][:, :])
                    nc.vector.tensor_mul(pa[t][:, :], ra_t[t][:, :], ra_t[t][:, :])
                dot = psum_all(pr, "dt")
                qq = psum_all(pq, "qq")
                aa = psum_all(pa, "aa")

                nq = pc.tile([32, 1], F32, tag="nq", name="nq")
                na = pc.tile([32, 1], F32, tag="na", name="na")
                nc.scalar.activation(nq[:, :], qq[:, :], AF.Sqrt)
                nc.scalar.activation(na[:, :], aa[:, :], AF.Sqrt)
                nc.vector.tensor_scalar_max(nq[:, :], nq[:, :], 1e-6)
                nc.vector.tensor_scalar_max(na[:, :], na[:, :], 1e-6)
                den = pc.tile([32, 1], F32, tag="den", name="den")
                nc.vector.tensor_mul(den[:, :], nq[:, :], na[:, :])
                rden = pc.tile([32, 1], F32, tag="rden", name="rden")
                nc.vector.reciprocal(rden[:, :], den[:, :])
                res = pc.tile([32, 1], F32, tag="res", name="res")
                nc.vector.tensor_mul(res[:, :], dot[:, :], rden[:, :])
                nc.gpsimd.dma_start(out_d[:].rearrange("(a b) -> a b", b=1),
                                    res[:, :])

    nc.finalize()
    return nc


def _prep(question, answer, Wq, bq, Wa, ba, U):
    bf = ml_dtypes.bfloat16
    qs = question.reshape(N_CORES, BS, M, E)
    as_ = answer.reshape(N_CORES, BS, L, E)

    def enc_z8(x, T):
        # x: [BS, T, E] -> Z^T rows [BS, 8, 128, T] bf16 (ctx shifts baked in)
        xt = x.transpose(0, 2, 1)  # [BS, E, T]
        xtp = np.zeros((x.shape[0], E, T + 2), np.float32)
        xtp[:, :, 1:T + 1] = xt
        z = np.zeros((x.shape[0], 1024, T), dtype=bf)
        for i in range(3):
            z[:, i * E:(i + 1) * E, :] = xtp[:, :, i:i + T].astype(bf)
        return z.reshape(x.shape[0], 8, 128, T)

    def enc_xq8(x):
        # [BS, M, E] -> [8, 128, BS*M] bf16
        z = enc_z8(x, M)  # [BS, 8, 128, 40]
        return np.ascontiguousarray(z.transpose(1, 2, 0, 3)).reshape(8, 128, MT)

    def enc_w8(W):
        # W [C, 900] -> W^T padded [8, 128, CP] bf16
        o = np.zeros((1024, CP), dtype=bf)
        o[0:900, 0:C] = W.T.astype(bf)
        return o.reshape(8, 128, CP)

    up = np.zeros((CP, CP), dtype=bf)
    up[0:C, 0:C] = U.astype(bf)
    up = up.reshape(NT, 128, CP)

    def enc_b(b):
        o = np.zeros((CP,), np.float32)
        o[0:C] = b
        return np.ascontiguousarray(o.reshape(NT, 128).T)

    com = {
        "wqt": enc_w8(Wq), "wat": enc_w8(Wa), "ut": up,
        "bq": enc_b(bq), "ba": enc_b(ba),
        "ident": np.eye(128, dtype=np.float32),
        "ones": np.ones((1, 128), dtype=bf),
        "ones_col": np.ones((128, 1), dtype=bf),
    }
    maps = []
    for i in range(N_CORES):
        m = dict(com)
        m["xq"] = enc_xq8(qs[i])
        m["xa"] = enc_z8(as_[i], L)
        maps.append(m)
    return maps


def kernel(question, answer, Wq, bq, Wa, ba, U, _trace=False):
    if "nc" not in _CACHE:
        _CACHE["nc"] = _build()
    nc = _CACHE["nc"]
    maps = _prep(np.asarray(question), np.asarray(answer), np.asarray(Wq),
                 np.asarray(bq), np.asarray(Wa), np.asarray(ba), np.asarray(U))
    r = run_bass_kernel_spmd(nc, maps, list(range(N_CORES)), trace=_trace)
    _CACHE["last"] = r
    return np.concatenate([r.results[i]["out"] for i in range(N_CORES)])



# revision 6
# speedup vs baseline: 1.0014x; 1.0014x over previous
"""AttentivePoolingNetwork Trainium2 kernel.

B=256 batch sharded 32/core across 8 NeuronCores. Per core:
  Q = cnn_encode(question)   [C=400(pad512), 32*40]   (bf16 matmuls, f32 psum)
  A = cnn_encode(answer)     [C=400(pad512), 32*400]  (kept in SBUF, bf16)
  P = U^T Q                  [C, 32*40]
  Gpre_b = P_b^T A_b         [40, 400] per batch item (tanh deferred)
  gq[b,m] = max_l Gpre, ga[b,l] = max_m Gpre (tanh applied after max)
  Softmax over the global batch dim via AllReduce(add) of local exp-sums,
  split in two (batches 0-27 / 28-31) so the first collective's latency
  hides under the tail of phase B.
  exp(tanh(ga)) rows are broadcast (GpSimd partition_broadcast) and
  multiplied into A during phase B; post-collective work is the 1/S scale
  and a free-dim reduce per c-tile, split across DVE/ACT/GpSimd:
  rQ = Q w_q, rA = A w_a, out = cos(rQ, rA)
"""

import numpy as np
import ml_dtypes

import concourse.bass as bass
import concourse.tile as tile
from concourse import bacc, mybir
import concourse.bass_isa as bass_isa
from concourse.bass_utils import run_bass_kernel_spmd

F32 = mybir.dt.float32
BF16 = mybir.dt.bfloat16
AF = mybir.ActivationFunctionType
OP = mybir.AluOpType

N_CORES = 8
B, M, L, E, C = 256, 40, 400, 300, 400
BS = B // N_CORES          # 32 batch per core
EP = 384                   # E padded to 3*128
CP = 512                   # C padded to 4*128
NT = CP // 128             # 4 c/d tiles
MT = BS * M                # 1280
LT = BS * L                # 12800
CHUNK = 8                  # batch chunk for Q/P matmuls (free dim 320)
NCH = BS // CHUNK          # 4
GRP = 4                    # batch group for the e-broadcast Y multiply
NG = BS // GRP             # 8 groups
PRE = 6                    # xa prefetch depth (batches in flight)
SPLIT = 28                 # batches 0..SPLIT-1 go in the early AllReduce

_CACHE = {}


def _build():
    nc = bacc.Bacc("TRN2", target_bir_lowering=False)

    xq_d = nc.dram_tensor("xq", [8, 128, MT], BF16, kind="ExternalInput")
    xa_d = nc.dram_tensor("xa", [BS, 8, 128, 400], BF16, kind="ExternalInput")
    wq_d = nc.dram_tensor("wqt", [8, 128, CP], BF16, kind="ExternalInput")
    wa_d = nc.dram_tensor("wat", [8, 128, CP], BF16, kind="ExternalInput")
    u_d = nc.dram_tensor("ut", [NT, 128, CP], BF16, kind="ExternalInput")
    bq_d = nc.dram_tensor("bq", [128, NT], F32, kind="ExternalInput")
    ba_d = nc.dram_tensor("ba", [128, NT], F32, kind="ExternalInput")
    id_d = nc.dram_tensor("ident", [128, 128], F32, kind="ExternalInput")
    on_d = nc.dram_tensor("ones", [1, 128], BF16, kind="ExternalInput")
    oc_d = nc.dram_tensor("ones_col", [128, 1], BF16, kind="ExternalInput")
    out_d = nc.dram_tensor("out", [32], F32, kind="ExternalOutput")

    with tile.TileContext(nc) as tc:
        with tc.tile_pool(name="const", bufs=1) as cp, \
             tc.tile_pool(name="dram", bufs=1, space="DRAM") as dp:
            # ---- persistent SBUF tensors ----
            wa_sb = cp.tile([128, 8 * CP], BF16, tag="wa_sb", name="wa_sb")
            bq_sb = cp.tile([128, NT], F32, tag="bq_sb", name="bq_sb")
            ba_sb = cp.tile([128, NT], F32, tag="ba_sb", name="ba_sb")
            id_sb = cp.tile([128, 128], F32, tag="id_sb", name="id_sb")
            on_sb = cp.tile([1, 128], BF16, tag="on_sb", name="on_sb")
            q_sb = [cp.tile([128, MT], BF16, tag=f"q_sb{t}", name=f"q_sb{t}") for t in range(NT)]
            a_sb = [cp.tile([128, LT], BF16, tag=f"a_sb{t}", name=f"a_sb{t}") for t in range(NT)]
            gq_all = cp.tile([40, BS], F32, tag="gq_all", name="gq_all")
            s_acc = [cp.tile([1, 400], F32, tag=f"s_acc{i}", name=f"s_acc{i}") for i in range(2)]
            sq_acc = [cp.tile([40, 1], F32, tag=f"sq_acc{i}", name=f"sq_acc{i}") for i in range(2)]
            rq_t = [cp.tile([128, BS], F32, tag=f"rq{t}", name=f"rq{t}") for t in range(NT)]
            ra_t = [cp.tile([128, BS], F32, tag=f"ra{t}", name=f"ra{t}") for t in range(NT)]

            ccin = [dp.tile([1, 440], F32, tag=f"ccin{i}", name=f"ccin{i}") for i in range(2)]
            ccout = [dp.tile([1, 440], F32, tag=f"ccout{i}", name=f"ccout{i}") for i in range(2)]
            ccwi = dp.tile([1, 8], F32, tag="ccwi", name="ccwi")
            ccwo = dp.tile([1, 8], F32, tag="ccwo", name="ccwo")

            # ---- load constants/inputs ----
            _pab_cm = tc.tile_pool(name="pab", bufs=1)
            pab_pool = _pab_cm.__enter__()
            p_sb = [pab_pool.tile([128, MT], BF16, tag=f"p_sb{t}", name=f"p_sb{t}") for t in range(NT)]
            _xap_cm = tc.tile_pool(name="xa_pool", bufs=PRE)
            xap = _xap_cm.__enter__()
            _pa_cm = tc.tile_pool(name="pa", bufs=1)
            pa_pool = _pa_cm.__enter__()
            xq_all = pa_pool.tile([128, 8 * MT], BF16, tag="xq_all", name="xq_all")
            wq_sb = pa_pool.tile([128, 8 * CP], BF16, tag="wq_sb", name="wq_sb")
            u_sb = pa_pool.tile([128, NT * CP], BF16, tag="u_sb", name="u_sb")
            # per-k-tile loads: the first Q matmul only waits for slice 0
            for k in range(8):
                nc.sync.dma_start(wq_sb[:, k * CP:(k + 1) * CP], wq_d[k])
                nc.sync.dma_start(xq_all[:, k * MT:(k + 1) * MT], xq_d[k])
            nc.sync.dma_start(bq_sb[:], bq_d[:])
            nc.scalar.dma_start(
                u_sb[:].rearrange("p (k d) -> p k d", k=NT),
                u_d[:].rearrange("k p d -> p k d"))
            nc.scalar.dma_start(
                wa_sb[:].rearrange("p (k c) -> p k c", k=8),
                wa_d[:].rearrange("k p c -> p k c"))
            nc.scalar.dma_start(ba_sb[:], ba_d[:])
            nc.sync.dma_start(id_sb[:], id_d[:])
            nc.sync.dma_start(on_sb[:], on_d[:])

            # xa prefetch for the first PRE batches
            xts = {}

            def fetch(bb):
                xt = xap.tile([128, 8 * 400], BF16, tag="xa_t", name="xa_t")
                nc.scalar.dma_start(
                    xt[:].rearrange("p (k l) -> p k l", k=8),
                    xa_d[bb].rearrange("k p l -> p k l"))
                xts[bb] = xt

            for bb in range(PRE):
                fetch(bb)

            # warm the ACT LUT table set (exp_and_others: copy/identity/
            # tanh/exp) so no demand load lands on the per-batch chain
            warm = cp.tile([1, 32], F32, tag="warm", name="warm")
            nc.vector.memset(warm[:, :], 0.25)
            nc.scalar.activation(warm[:, :], warm[:, :], AF.Tanh)
            nc.scalar.activation(warm[:, :], warm[:, :], AF.Exp)

            # dummy collective: pre-pays the CC mesh wakeup and absorbs the
            # cross-core launch skew long before the real AllReduce
            wsync = cp.tile([1, 8], F32, tag="wsync", name="wsync")
            nc.vector.memset(wsync[:, :], 1.0)
            nc.gpsimd.dma_start(ccwi[0:1, :], wsync[:, :])
            nc.gpsimd.collective_compute(
                "AllReduce", OP.add,
                replica_groups=[list(range(N_CORES))],
                ins=[ccwi[:].opt()], outs=[ccwo[:].opt()])

            nc.vector.memset(s_acc[0][:, :], 0.0)
            nc.vector.memset(s_acc[1][:, :], 0.0)
            nc.vector.memset(sq_acc[0][:, :], 0.0)
            nc.vector.memset(sq_acc[1][:, :], 0.0)

            # ---- Phase A: Q encode + P = U^T Q, chunk-pipelined ----
            with tc.tile_pool(name="qpsum", bufs=3, space="PSUM") as qp, \
                 tc.tile_pool(name="ppsum", bufs=2, space="PSUM") as pp:
                for s in range(NCH):
                    sl = slice(s * CHUNK * M, (s + 1) * CHUNK * M)
                    for t in range(NT):
                        ps = qp.tile([128, CHUNK * M], F32, tag="qps", name="qps")
                        for k in range(8):
                            lhsT = wq_sb[:, k * CP + t * 128:k * CP + (t + 1) * 128]
                            nc.tensor.matmul(ps[:, :], lhsT, xq_all[:, k * MT + s * CHUNK * M:
                                                                    k * MT + (s + 1) * CHUNK * M],
                                             start=(k == 0), stop=(k == 7))
                        nc.scalar.activation(q_sb[t][:, sl], ps[:, :], AF.Identity,
                                             bias=bq_sb[:, t:t + 1])
                    for t in range(NT):
                        ps = pp.tile([128, CHUNK * M], F32, tag="pps", name="pps")
                        for kk in range(NT):
                            lhsT = u_sb[:, kk * CP + t * 128:kk * CP + (t + 1) * 128]
                            nc.tensor.matmul(ps[:, :], lhsT, q_sb[kk][:, sl],
                                             start=(kk == 0), stop=(kk == NT - 1))
                        nc.vector.tensor_copy(p_sb[t][:, sl], ps[:, :])

            _pa_cm.__exit__(None, None, None)

            # ---- Phase B: per-batch A encode + G + pooled maxes + exp ----
            with tc.tile_pool(name="apsum", bufs=6, space="PSUM") as ap, \
                 tc.tile_pool(name="gpsum", bufs=2, space="PSUM") as gp, \
                 tc.tile_pool(name="ebc", bufs=2) as ebcp, \
                 tc.tile_pool(name="rows", bufs=3) as rowp, \
                 tc.tile_pool(name="tree", bufs=2) as trp:

                e_grp = {}

                def do_g(bb):
                    acc = 0 if bb < SPLIT else 1
                    g = gp.tile([64, 400], F32, tag="gps", name="gps")
                    for kk in range(NT):
                        nc.tensor.matmul(
                            g[0:40, :],
                            p_sb[kk][:, bb * M:(bb + 1) * M],
                            a_sb[kk][:, bb * L:(bb + 1) * L],
                            start=(kk == 0), stop=(kk == NT - 1))
                    nc.vector.reduce_max(gq_all[0:40, bb:bb + 1], g[0:40, :],
                                         axis=mybir.AxisListType.X, op=OP.max)
                    # incremental gq-side softmax sum
                    e1q = rowp.tile([40, 1], F32, tag="e1q", name="e1q")
                    nc.scalar.activation(e1q[:, :], gq_all[0:40, bb:bb + 1],
                                         AF.Tanh)
                    nc.scalar.activation(e1q[:, :], e1q[:, :], AF.Exp)
                    nc.vector.tensor_add(sq_acc[acc][:, :], sq_acc[acc][:, :], e1q[:, :])
                    g_s = trp.tile([40, 400], F32, tag="g_s", name="g_s")
                    nc.scalar.activation(g_s[:, :], g[0:40, :], AF.Copy)
                    g_r = trp.tile([40, 400], F32, tag="g_r", name="g_r")
                    nc.gpsimd.partition_all_reduce(
                        g_r[:, :], g_s[:, :], channels=40,
                        reduce_op=bass_isa.ReduceOp.max)
                    # exp(tanh(max_m G)) row for batch bb
                    t1 = rowp.tile([1, 400], F32, tag="t1", name="t1")
                    nc.scalar.activation(t1[:, :], g_r[0:1, :], AF.Tanh)
                    e1 = rowp.tile([1, 400], F32, tag="e1", name="e1")
                    nc.scalar.activation(e1[:, :], t1[:, :], AF.Exp)
                    nc.vector.tensor_add(s_acc[acc][:, :], s_acc[acc][:, :], e1[:, :])
                    # broadcast exp row to 128 partitions for the Y multiply
                    e1b = rowp.tile([1, 400], BF16, tag="e1b", name="e1b")
                    nc.vector.tensor_copy(e1b[:, :], e1[:, :])
                    eg = e_grp[bb // GRP]
                    nc.gpsimd.partition_broadcast(
                        eg[:, (bb % GRP) * 400:(bb % GRP + 1) * 400], e1b[:, :])

                def tail(grp):
                    # Y = A .* exp-broadcast, in place over this group's slice
                    eg = e_grp.pop(grp)
                    sl = slice(grp * GRP * L, (grp + 1) * GRP * L)
                    for t in range(NT):
                        nc.vector.tensor_tensor(a_sb[t][:, sl], a_sb[t][:, sl],
                                                eg[:, :], op=OP.mult)

                for bb in range(BS):
                    if bb + PRE < BS:
                        fetch(bb + PRE)
                    if bb % GRP == 0:
                        e_grp[bb // GRP] = ebcp.tile([128, GRP * 400], BF16,
                                                     tag="e_g", name="e_g")
                    xt = xts.pop(bb)
                    for t in range(NT):
                        aps = ap.tile([128, 400], F32, tag="aps", name="aps")
                        for k in range(8):
                            lhsT = wa_sb[:, k * CP + t * 128:k * CP + (t + 1) * 128]
                            nc.tensor.matmul(aps[:, :], lhsT, xt[:, k * 400:(k + 1) * 400],
                                             start=(k == 0), stop=(k == 7))
                        nc.scalar.activation(a_sb[t][:, bb * L:(bb + 1) * L],
                                             aps[:, :], AF.Identity,
                                             bias=ba_sb[:, t:t + 1])
                    do_g(bb)
                    if bb % GRP == GRP - 1:
                        tail(bb // GRP)
                    if bb == SPLIT - 1:
                        # early collective over batches 0..SPLIT-1: latency
                        # hides under the remaining batches' encode work
                        nc.gpsimd.dma_start(ccin[0][0:1, 0:40], sq_acc[0][:, :])
                        nc.gpsimd.dma_start(ccin[0][0:1, 40:440], s_acc[0][:, :])
                        nc.gpsimd.collective_compute(
                            "AllReduce", OP.add,
                            replica_groups=[list(range(N_CORES))],
                            ins=[ccin[0][:].opt()], outs=[ccout[0][:].opt()])

            _xap_cm.__exit__(None, None, None)
            _pab_cm.__exit__(None, None, None)

            # ---- Phase C: batch softmax (AllReduce #2) + pooled sums ----
            with tc.tile_pool(name="phc", bufs=1) as pc, \
                 tc.tile_pool(name="cpsum", bufs=2, space="PSUM") as cps, \
                 tc.tile_pool(name="cpsum1", bufs=2, space="PSUM") as cp1:
                nc.gpsimd.dma_start(ccin[1][0:1, 0:40], sq_acc[1][:, :])
                nc.gpsimd.dma_start(ccin[1][0:1, 40:440], s_acc[1][:, :])
                nc.gpsimd.collective_compute(
                    "AllReduce", OP.add,
                    replica_groups=[list(range(N_CORES))],
                    ins=[ccin[1][:].opt()], outs=[ccout[1][:].opt()])

                # --- work hidden under the collective: Yq = Q .* exp_q ---
                tq = pc.tile([40, BS], F32, tag="tq", name="tq")
                nc.scalar.activation(tq[:, :], gq_all[:, :], AF.Tanh)
                e_q = pc.tile([40, BS], F32, tag="e_q", name="e_q")
                nc.scalar.activation(e_q[:, :], tq[:, :], AF.Exp)
                eqt_ps = cp1.tile([BS, 40], F32, tag="c1", name="eqt_ps")
                nc.tensor.transpose(eqt_ps[:, :], e_q[:, :], id_sb[0:40, 0:40])
                eqt = pc.tile([BS, 40], BF16, tag="eqt", name="eqt")
                nc.vector.tensor_copy(eqt[:, :], eqt_ps[:, :])
                eq_fl = pc.tile([1, MT], BF16, tag="eq_fl", name="eq_fl")
                nc.gpsimd.dma_start(eq_fl[0:1, :], eqt[:, :])
                eq_bc = pc.tile([128, MT], BF16, tag="eq_bc", name="eq_bc")
                nc.gpsimd.partition_broadcast(eq_bc[:, :], eq_fl[0:1, :])
                for t in range(NT):
                    nc.vector.tensor_tensor(q_sb[t][:, :], q_sb[t][:, :],
                                            eq_bc[:, :], op=OP.mult)

                # --- post-collective: u = 1/S, scale + fused reduces ---
                ss0 = pc.tile([1, 440], F32, tag="ss0", name="ss0")
                ss1 = pc.tile([1, 440], F32, tag="ss1", name="ss1")
                nc.gpsimd.dma_start(ss0[:, :], ccout[0][0:1, :])
                nc.gpsimd.dma_start(ss1[:, :], ccout[1][0:1, :])
                ss = pc.tile([1, 440], F32, tag="ss", name="ss")
                nc.vector.tensor_add(ss[:, :], ss0[:, :], ss1[:, :])
                ur = pc.tile([1, 440], F32, tag="ur", name="ur")
                nc.vector.reciprocal_approx_fast(ur[:, :], ss[:, :])
                ub = pc.tile([1, 440], BF16, tag="ub", name="ub")
                nc.vector.tensor_copy(ub[:, :], ur[:, :])
                ua_bc = pc.tile([128, 400], BF16, tag="ua_bc", name="ua_bc")
                nc.gpsimd.partition_broadcast(ua_bc[:, :], ub[0:1, 40:440])
                uq_bc = pc.tile([128, 40], BF16, tag="uq_bc", name="uq_bc")
                nc.gpsimd.partition_broadcast(uq_bc[:, :], ub[0:1, 0:40])

                ua_v = ua_bc[:].rearrange("p (o l) -> p o l", o=1).broadcast_to((128, BS, 400))
                uq_v = uq_bc[:].rearrange("p (o m) -> p o m", o=1).broadcast_to((128, BS, 40))

                def tree_sum(eng, av, out, n):
                    # pairwise in-place halving keeps every op all-bf16
                    # (2x rate); a strided X-reduce would run at 1x.
                    while n > 25 and n % 2 == 0:
                        h = n // 2
                        eng.tensor_tensor(av[:, :, 0:h], av[:, :, 0:h],
                                          av[:, :, h:n], op=OP.add)
                        n = h
                    # free-axis reduce is DVE-only
                    nc.vector.reduce_sum(out, av[:, :, 0:n],
                                         axis=mybir.AxisListType.X, op=OP.add)

                scr = pc.tile([128, 400], BF16, tag="scr", name="scr")
                for t in range(NT):
                    av = a_sb[t][:].rearrange("p (b l) -> p b l", b=BS)
                    nc.vector.tensor_tensor(av, av, ua_v, op=OP.mult)
                    if t == 0:
                        # otherwise-idle scalar engine reduces one c-tile
                        for bb in range(BS):
                            nc.scalar.activation(
                                scr[:, :], av[:, bb, :], AF.Copy,
                                accum_out=ra_t[t][:, bb:bb + 1])
                    elif t == 1:
                        # gpsimd takes another (slow but parallel)
                        tree_sum(nc.gpsimd, av, ra_t[t][:, :], 400)
                    else:
                        tree_sum(nc.vector, av, ra_t[t][:, :], 400)
                for t in range(NT):
                    qv = q_sb[t][:].rearrange("p (b m) -> p b m", b=BS)
                    nc.vector.tensor_tensor(qv, qv, uq_v, op=OP.mult)
                    tree_sum(nc.vector, qv, rq_t[t][:, :], 40)

                # cosine similarity: reduce over c = 4 tiles x 128 partitions
                # via accumulating PE transposes: psum [BS,128] = sum_t P_t^T,
                # then a free-dim reduce gives the per-b column.
                def psum_all(tiles, tag):
                    tps = cps.tile([BS, 128], F32, tag="cts", name=f"{tag}tp")
                    for t in range(NT):
                        nc.tensor.matmul(tps[:, :], tiles[t][:, :], id_sb[:, :],
                                         is_transpose=True,
                                         start=(t == 0), stop=(t == NT - 1))
                    col = pc.tile([32, 1], F32, tag=f"{tag}c", name=f"{tag}c")
                    nc.vector.reduce_sum(col[:, :], tps[:, :],
                                         axis=mybir.AxisListType.X, op=OP.add)
                    return col

                pr = [pc.tile([128, BS], F32, tag=f"pr{t}", name=f"pr{t}") for t in range(NT)]
                pq = [pc.tile([128, BS], F32, tag=f"pq{t}", name=f"pq{t}") for t in range(NT)]
                pa = [pc.tile([128, BS], F32, tag=f"pa{t}", name=f"pa{t}") for t in range(NT)]
                for t in range(NT):
                    nc.vector.tensor_mul(pr[t][:, :], rq_t[t][:, :], ra_t[t][:, :])
                    nc.vector.tensor_mul(pq[t][:, :], rq_t[t][:, :], rq_t[t][:, :])
                    nc.vector.tensor_mul(pa[t][:, :], ra_t[t][:, :], ra_t[t][:, :])
                dot = psum_all(pr, "dt")
                qq = psum_all(pq, "qq")
                aa = psum_all(pa, "aa")

                nq = pc.tile([32, 1], F32, tag="nq", name="nq")
                na = pc.tile([32, 1], F32, tag="na", name="na")
                nc.scalar.activation(nq[:, :], qq[:, :], AF.Sqrt)
                nc.scalar.activation(na[:, :], aa[:, :], AF.Sqrt)
                nc.vector.tensor_scalar_max(nq[:, :], nq[:, :], 1e-6)
                nc.vector.tensor_scalar_max(na[:, :], na[:, :], 1e-6)
                den = pc.tile([32, 1], F32, tag="den", name="den")
                nc.vector.tensor_mul(den[:, :], nq[:, :], na[:, :])
                rden = pc.tile([32, 1], F32, tag="rden", name="rden")
                nc.vector.reciprocal(rden[:, :], den[:, :])
                res = pc.tile([32, 1], F32, tag="res", name="res")
                nc.vector.tensor_mul(res[:, :], dot[:, :], rden[:, :])
                nc.gpsimd.dma_start(out_d[:].rearrange("(a b) -> a b", b=1),
                                    res[:, :])

    nc.finalize()
    return nc


def _prep(question, answer, Wq, bq, Wa, ba, U):
    bf = ml_dtypes.bfloat16
    qs = question.reshape(N_CORES, BS, M, E)
    as_ = answer.reshape(N_CORES, BS, L, E)

    def enc_z8(x, T):
        # x: [BS, T, E] -> Z^T rows [BS, 8, 128, T] bf16 (ctx shifts baked in)
        xt = x.transpose(0, 2, 1)  # [BS, E, T]
        xtp = np.zeros((x.shape[0], E, T + 2), np.float32)
        xtp[:, :, 1:T + 1] = xt
        z = np.zeros((x.shape[0], 1024, T), dtype=bf)
        for i in range(3):
            z[:, i * E:(i + 1) * E, :] = xtp[:, :, i:i + T].astype(bf)
        return z.reshape(x.shape[0], 8, 128, T)

    def enc_xq8(x):
        # [BS, M, E] -> [8, 128, BS*M] bf16
        z = enc_z8(x, M)  # [BS, 8, 128, 40]
        return np.ascontiguousarray(z.transpose(1, 2, 0, 3)).reshape(8, 128, MT)

    def enc_w8(W):
        # W [C, 900] -> W^T padded [8, 128, CP] bf16
        o = np.zeros((1024, CP), dtype=bf)
        o[0:900, 0:C] = W.T.astype(bf)
        return o.reshape(8, 128, CP)

    up = np.zeros((CP, CP), dtype=bf)
    up[0:C, 0:C] = U.astype(bf)
    up = up.reshape(NT, 128, CP)

    def enc_b(b):
        o = np.zeros((CP,), np.float32)
        o[0:C] = b
        return np.ascontiguousarray(o.reshape(NT, 128).T)

    com = {
        "wqt": enc_w8(Wq), "wat": enc_w8(Wa), "ut": up,
        "bq": enc_b(bq), "ba": enc_b(ba),
        "ident": np.eye(128, dtype=np.float32),
        "ones": np.ones((1, 128), dtype=bf),
        "ones_col": np.ones((128, 1), dtype=bf),
    }
    maps = []
    for i in range(N_CORES):
        m = dict(com)
        m["xq"] = enc_xq8(qs[i])
        m["xa"] = enc_z8(as_[i], L)
        maps.append(m)
    return maps


def kernel(question, answer, Wq, bq, Wa, ba, U, _trace=False):
    if "nc" not in _CACHE:
        _CACHE["nc"] = _build()
    nc = _CACHE["nc"]
    maps = _prep(np.asarray(question), np.asarray(answer), np.asarray(Wq),
                 np.asarray(bq), np.asarray(Wa), np.asarray(ba), np.asarray(U))
    r = run_bass_kernel_spmd(nc, maps, list(range(N_CORES)), trace=_trace)
    _CACHE["last"] = r
    return np.concatenate([r.results[i]["out"] for i in range(N_CORES)])


# revision 8
# speedup vs baseline: 1.0768x; 1.0753x over previous
"""AttentivePoolingNetwork Trainium2 kernel.

B=256 batch sharded 32/core across 8 NeuronCores. Per core:
  Q = cnn_encode(question)   [C=400(pad512), 32*40]   (bf16 matmuls, f32 psum)
  A = cnn_encode(answer)     [C=400(pad512), 32*400]  (kept in SBUF, bf16)
  P = U^T Q                  [C, 32*40]
  Gpre_b = P_b^T A_b         [40, 400] per batch item (tanh deferred)
  gq[b,m] = max_l Gpre, ga[b,l] = max_m Gpre (tanh applied after max)
  Softmax over the global batch dim via AllReduce(add) of local exp-sums,
  split in two (batches 0-27 / 28-31) so the first collective's latency
  hides under the tail of phase B.
  exp(tanh(ga)) rows are broadcast (GpSimd partition_broadcast) and
  multiplied into A during phase B; post-collective work is the 1/S scale
  and a free-dim reduce per c-tile, split across DVE/ACT/GpSimd:
  rQ = Q w_q, rA = A w_a, out = cos(rQ, rA)
"""

import numpy as np
import ml_dtypes

import concourse.bass as bass
import concourse.tile as tile
from concourse import bacc, mybir
import concourse.bass_isa as bass_isa
from concourse.bass_utils import run_bass_kernel_spmd

F32 = mybir.dt.float32
BF16 = mybir.dt.bfloat16
AF = mybir.ActivationFunctionType
OP = mybir.AluOpType

N_CORES = 8
B, M, L, E, C = 256, 40, 400, 300, 400
BS = B // N_CORES          # 32 batch per core
EP = 384                   # E padded to 3*128
CP = 512                   # C padded to 4*128
NT = CP // 128             # 4 c/d tiles
MT = BS * M                # 1280
LT = BS * L                # 12800
CHUNK = 8                  # batch chunk for Q/P matmuls (free dim 320)
NCH = BS // CHUNK          # 4
GRP = 4                    # batch group for the e-broadcast Y multiply
NG = BS // GRP             # 8 groups
PRE = 6                    # xa prefetch depth (batches in flight)
SPLIT = 28                 # batches 0..SPLIT-1 go in the early AllReduce

_CACHE = {}


def _build():
    nc = bacc.Bacc("TRN2", target_bir_lowering=False)

    xq_d = nc.dram_tensor("xq", [8, 128, MT], BF16, kind="ExternalInput")
    xa_d = nc.dram_tensor("xa", [BS, 8, 128, 400], BF16, kind="ExternalInput")
    wq_d = nc.dram_tensor("wqt", [8, 128, CP], BF16, kind="ExternalInput")
    wa_d = nc.dram_tensor("wat", [8, 128, CP], BF16, kind="ExternalInput")
    u_d = nc.dram_tensor("ut", [NT, 128, CP], BF16, kind="ExternalInput")
    bq_d = nc.dram_tensor("bq", [128, NT], F32, kind="ExternalInput")
    ba_d = nc.dram_tensor("ba", [128, NT], F32, kind="ExternalInput")
    id_d = nc.dram_tensor("ident", [128, 128], F32, kind="ExternalInput")
    on_d = nc.dram_tensor("ones", [1, 128], BF16, kind="ExternalInput")
    oc_d = nc.dram_tensor("ones_col", [128, 1], BF16, kind="ExternalInput")
    out_d = nc.dram_tensor("out", [32], F32, kind="ExternalOutput")

    with tile.TileContext(nc) as tc:
        with tc.tile_pool(name="const", bufs=1) as cp, \
             tc.tile_pool(name="dram", bufs=1, space="DRAM") as dp:
            # ---- persistent SBUF tensors ----
            wa_sb = cp.tile([128, 8 * CP], BF16, tag="wa_sb", name="wa_sb")
            bq_sb = cp.tile([128, NT], F32, tag="bq_sb", name="bq_sb")
            ba_sb = cp.tile([128, NT], F32, tag="ba_sb", name="ba_sb")
            id_sb = cp.tile([128, 128], F32, tag="id_sb", name="id_sb")
            on_sb = cp.tile([1, 128], BF16, tag="on_sb", name="on_sb")
            q_sb = [cp.tile([128, MT], BF16, tag=f"q_sb{t}", name=f"q_sb{t}") for t in range(NT)]
            a_sb = [cp.tile([128, LT], BF16, tag=f"a_sb{t}", name=f"a_sb{t}") for t in range(NT)]
            gq_all = cp.tile([40, BS], F32, tag="gq_all", name="gq_all")
            s_acc = [cp.tile([1, 400], F32, tag=f"s_acc{i}", name=f"s_acc{i}") for i in range(2)]
            sq_acc = [cp.tile([40, 1], F32, tag=f"sq_acc{i}", name=f"sq_acc{i}") for i in range(2)]
            rq_t = [cp.tile([128, BS], F32, tag=f"rq{t}", name=f"rq{t}") for t in range(NT)]
            ra_t = [cp.tile([128, BS], F32, tag=f"ra{t}", name=f"ra{t}") for t in range(NT)]

            ccin = [dp.tile([1, 440], F32, tag=f"ccin{i}", name=f"ccin{i}") for i in range(2)]
            ccout = [dp.tile([8, 440], F32, tag=f"ccout{i}", name=f"ccout{i}") for i in range(2)]
            ccwi = dp.tile([1, 8], F32, tag="ccwi", name="ccwi")
            ccwo = dp.tile([1, 8], F32, tag="ccwo", name="ccwo")

            # ---- load constants/inputs ----
            _pab_cm = tc.tile_pool(name="pab", bufs=1)
            pab_pool = _pab_cm.__enter__()
            p_sb = [pab_pool.tile([128, MT], BF16, tag=f"p_sb{t}", name=f"p_sb{t}") for t in range(NT)]
            _xap_cm = tc.tile_pool(name="xa_pool", bufs=PRE)
            xap = _xap_cm.__enter__()
            _pa_cm = tc.tile_pool(name="pa", bufs=1)
            pa_pool = _pa_cm.__enter__()
            xq_all = pa_pool.tile([128, 8 * MT], BF16, tag="xq_all", name="xq_all")
            wq_sb = pa_pool.tile([128, 8 * CP], BF16, tag="wq_sb", name="wq_sb")
            u_sb = pa_pool.tile([128, NT * CP], BF16, tag="u_sb", name="u_sb")
            # per-k-tile loads: the first Q matmul only waits for slice 0
            for k in range(8):
                nc.sync.dma_start(wq_sb[:, k * CP:(k + 1) * CP], wq_d[k])
                nc.sync.dma_start(xq_all[:, k * MT:(k + 1) * MT], xq_d[k])
            nc.sync.dma_start(bq_sb[:], bq_d[:])
            nc.scalar.dma_start(
                u_sb[:].rearrange("p (k d) -> p k d", k=NT),
                u_d[:].rearrange("k p d -> p k d"))
            nc.scalar.dma_start(
                wa_sb[:].rearrange("p (k c) -> p k c", k=8),
                wa_d[:].rearrange("k p c -> p k c"))
            nc.scalar.dma_start(ba_sb[:], ba_d[:])
            nc.sync.dma_start(id_sb[:], id_d[:])
            nc.sync.dma_start(on_sb[:], on_d[:])

            # xa prefetch for the first PRE batches
            xts = {}

            def fetch(bb):
                xt = xap.tile([128, 8 * 400], BF16, tag="xa_t", name="xa_t")
                nc.scalar.dma_start(
                    xt[:].rearrange("p (k l) -> p k l", k=8),
                    xa_d[bb].rearrange("k p l -> p k l"))
                xts[bb] = xt

            for bb in range(PRE):
                fetch(bb)

            # warm the ACT LUT table set (exp_and_others: copy/identity/
            # tanh/exp) so no demand load lands on the per-batch chain
            warm = cp.tile([1, 32], F32, tag="warm", name="warm")
            nc.vector.memset(warm[:, :], 0.25)
            nc.scalar.activation(warm[:, :], warm[:, :], AF.Tanh)
            nc.scalar.activation(warm[:, :], warm[:, :], AF.Exp)

            # dummy collective: pre-pays the CC mesh wakeup and absorbs the
            # cross-core launch skew long before the real AllReduce
            wsync = cp.tile([1, 8], F32, tag="wsync", name="wsync")
            nc.vector.memset(wsync[:, :], 1.0)
            nc.gpsimd.dma_start(ccwi[0:1, :], wsync[:, :])
            nc.gpsimd.collective_compute(
                "AllReduce", OP.add,
                replica_groups=[list(range(N_CORES))],
                ins=[ccwi[:].opt()], outs=[ccwo[:].opt()])

            nc.vector.memset(s_acc[0][:, :], 0.0)
            nc.vector.memset(s_acc[1][:, :], 0.0)
            nc.vector.memset(sq_acc[0][:, :], 0.0)
            nc.vector.memset(sq_acc[1][:, :], 0.0)

            # ---- Phase A: Q encode + P = U^T Q ----
            # t-outer with 4 chunk-psums per t: each LDWEIGHTS serves 4
            # matmuls, keeping the PE stream dense
            with tc.tile_pool(name="qpsum", bufs=8, space="PSUM") as qp:
                for t in range(NT):
                    ps = [qp.tile([128, CHUNK * M], F32, tag="qps", name="qps") for _ in range(NCH)]
                    for k in range(8):
                        lhsT = wq_sb[:, k * CP + t * 128:k * CP + (t + 1) * 128]
                        for s in range(NCH):
                            rhs = xq_all[:, k * MT + s * CHUNK * M:
                                         k * MT + (s + 1) * CHUNK * M]
                            nc.tensor.matmul(ps[s][:, :], lhsT, rhs,
                                             start=(k == 0), stop=(k == 7))
                    for s in range(NCH):
                        nc.vector.tensor_add(
                            q_sb[t][:, s * CHUNK * M:(s + 1) * CHUNK * M],
                            ps[s][:, :],
                            bq_sb[:, t:t + 1].broadcast_to((128, CHUNK * M)))

            with tc.tile_pool(name="ppsum", bufs=8, space="PSUM") as pp:
                for t in range(NT):
                    for s in range(NCH):
                        ps = pp.tile([128, CHUNK * M], F32, tag="pps", name="pps")
                        for kk in range(NT):
                            lhsT = u_sb[:, kk * CP + t * 128:kk * CP + (t + 1) * 128]
                            rhs = q_sb[kk][:, s * CHUNK * M:(s + 1) * CHUNK * M]
                            nc.tensor.matmul(ps[:, :], lhsT, rhs,
                                             start=(kk == 0), stop=(kk == NT - 1))
                        nc.vector.tensor_copy(
                            p_sb[t][:, s * CHUNK * M:(s + 1) * CHUNK * M], ps[:, :])

            _pa_cm.__exit__(None, None, None)

            # ---- Phase B: per-batch A encode + G + pooled maxes + exp ----
            with tc.tile_pool(name="apsum", bufs=6, space="PSUM") as ap, \
                 tc.tile_pool(name="gpsum", bufs=2, space="PSUM") as gp, \
                 tc.tile_pool(name="ebc", bufs=2) as ebcp, \
                 tc.tile_pool(name="rows", bufs=3) as rowp, \
                 tc.tile_pool(name="tree", bufs=2) as trp:

                e_grp = {}

                def do_g(bb):
                    acc = 0 if bb < SPLIT else 1
                    g = gp.tile([64, 400], F32, tag="gps", name="gps")
                    for kk in range(NT):
                        nc.tensor.matmul(
                            g[0:40, :],
                            p_sb[kk][:, bb * M:(bb + 1) * M],
                            a_sb[kk][:, bb * L:(bb + 1) * L],
                            start=(kk == 0), stop=(kk == NT - 1))
                    nc.vector.reduce_max(gq_all[0:40, bb:bb + 1], g[0:40, :],
                                         axis=mybir.AxisListType.X, op=OP.max)
                    # incremental gq-side softmax sum
                    e1q = rowp.tile([40, 1], F32, tag="e1q", name="e1q")
                    nc.scalar.activation(e1q[:, :], gq_all[0:40, bb:bb + 1],
                                         AF.Tanh)
                    nc.scalar.activation(e1q[:, :], e1q[:, :], AF.Exp)
                    nc.vector.tensor_add(sq_acc[acc][:, :], sq_acc[acc][:, :], e1q[:, :])
                    g_s = trp.tile([40, 400], F32, tag="g_s", name="g_s")
                    nc.scalar.activation(g_s[:, :], g[0:40, :], AF.Copy)
                    g_r = trp.tile([40, 400], F32, tag="g_r", name="g_r")
                    nc.gpsimd.partition_all_reduce(
                        g_r[:, :], g_s[:, :], channels=40,
                        reduce_op=bass_isa.ReduceOp.max)
                    # exp(tanh(max_m G)) row for batch bb
                    t1 = rowp.tile([1, 400], F32, tag="t1", name="t1")
                    nc.scalar.activation(t1[:, :], g_r[0:1, :], AF.Tanh)
                    e1 = rowp.tile([1, 400], F32, tag="e1", name="e1")
                    nc.scalar.activation(e1[:, :], t1[:, :], AF.Exp)
                    nc.vector.tensor_add(s_acc[acc][:, :], s_acc[acc][:, :], e1[:, :])
                    # broadcast exp row to 128 partitions for the Y multiply
                    e1b = rowp.tile([1, 400], BF16, tag="e1b", name="e1b")
                    nc.vector.tensor_copy(e1b[:, :], e1[:, :])
                    eg = e_grp[bb // GRP]
                    nc.gpsimd.partition_broadcast(
                        eg[:, (bb % GRP) * 400:(bb % GRP + 1) * 400], e1b[:, :])

                def tail(grp):
                    # Y = A .* exp-broadcast, in place over this group's slice
                    eg = e_grp.pop(grp)
                    sl = slice(grp * GRP * L, (grp + 1) * GRP * L)
                    for t in range(NT):
                        nc.vector.tensor_tensor(a_sb[t][:, sl], a_sb[t][:, sl],
                                                eg[:, :], op=OP.mult)

                for bb in range(BS):
                    if bb + PRE < BS:
                        fetch(bb + PRE)
                    if bb % GRP == 0:
                        e_grp[bb // GRP] = ebcp.tile([128, GRP * 400], BF16,
                                                     tag="e_g", name="e_g")
                    xt = xts.pop(bb)
                    for t in range(NT):
                        aps = ap.tile([128, 400], F32, tag="aps", name="aps")
                        for k in range(8):
                            lhsT = wa_sb[:, k * CP + t * 128:k * CP + (t + 1) * 128]
                            nc.tensor.matmul(aps[:, :], lhsT, xt[:, k * 400:(k + 1) * 400],
                                             start=(k == 0), stop=(k == 7))
                        nc.scalar.activation(a_sb[t][:, bb * L:(bb + 1) * L],
                                             aps[:, :], AF.Identity,
                                             bias=ba_sb[:, t:t + 1])
                    do_g(bb)
                    if bb % GRP == GRP - 1:
                        tail(bb // GRP)
                    if bb == SPLIT - 1:
                        # early collective over batches 0..SPLIT-1: latency
                        # hides under the remaining batches' encode work
                        nc.gpsimd.dma_start(ccin[0][0:1, 0:40], sq_acc[0][:, :])
                        nc.gpsimd.dma_start(ccin[0][0:1, 40:440], s_acc[0][:, :])
                        nc.gpsimd.collective_compute(
                            "AllGather", OP.bypass,
                            replica_groups=[list(range(N_CORES))],
                            ins=[ccin[0][:].opt()], outs=[ccout[0][:].opt()])

            _xap_cm.__exit__(None, None, None)
            _pab_cm.__exit__(None, None, None)

            # ---- Phase C: batch softmax (AllReduce #2) + pooled sums ----
            with tc.tile_pool(name="phc", bufs=1) as pc, \
                 tc.tile_pool(name="cpsum", bufs=2, space="PSUM") as cps, \
                 tc.tile_pool(name="cpsum1", bufs=2, space="PSUM") as cp1:
                nc.gpsimd.dma_start(ccin[1][0:1, 0:40], sq_acc[1][:, :])
                nc.gpsimd.dma_start(ccin[1][0:1, 40:440], s_acc[1][:, :])
                nc.gpsimd.collective_compute(
                    "AllGather", OP.bypass,
                    replica_groups=[list(range(N_CORES))],
                    ins=[ccin[1][:].opt()], outs=[ccout[1][:].opt()])

                # --- work hidden under the collective: Yq = Q .* exp_q ---
                tq = pc.tile([40, BS], F32, tag="tq", name="tq")
                nc.scalar.activation(tq[:, :], gq_all[:, :], AF.Tanh)
                e_q = pc.tile([40, BS], F32, tag="e_q", name="e_q")
                nc.scalar.activation(e_q[:, :], tq[:, :], AF.Exp)
                eqt_ps = cp1.tile([BS, 40], F32, tag="c1", name="eqt_ps")
                nc.tensor.transpose(eqt_ps[:, :], e_q[:, :], id_sb[0:40, 0:40])
                eqt = pc.tile([BS, 40], BF16, tag="eqt", name="eqt")
                nc.vector.tensor_copy(eqt[:, :], eqt_ps[:, :])
                eq_fl = pc.tile([1, MT], BF16, tag="eq_fl", name="eq_fl")
                nc.gpsimd.dma_start(eq_fl[0:1, :], eqt[:, :])
                eq_bc = pc.tile([128, MT], BF16, tag="eq_bc", name="eq_bc")
                nc.gpsimd.partition_broadcast(eq_bc[:, :], eq_fl[0:1, :])
                for t in range(NT):
                    nc.vector.tensor_tensor(q_sb[t][:, :], q_sb[t][:, :],
                                            eq_bc[:, :], op=OP.mult)

                # --- post-collective: u = 1/S, scale + fused reduces ---
                sg0 = pc.tile([8, 440], F32, tag="sg0", name="sg0")
                sg1 = pc.tile([8, 440], F32, tag="sg1", name="sg1")
                nc.gpsimd.dma_start(sg0[:, :], ccout[0][:, :])
                nc.gpsimd.dma_start(sg1[:, :], ccout[1][:, :])
                sg = pc.tile([8, 440], F32, tag="sg", name="sg")
                nc.vector.tensor_add(sg[:, :], sg0[:, :], sg1[:, :])
                sgr = pc.tile([8, 440], F32, tag="sgr", name="sgr")
                nc.gpsimd.partition_all_reduce(sgr[:, :], sg[:, :], channels=8,
                                               reduce_op=bass_isa.ReduceOp.add)
                ss = sgr[0:1, :]
                ur = pc.tile([1, 440], F32, tag="ur", name="ur")
                nc.vector.reciprocal_approx_fast(ur[:, :], ss)
                ub = pc.tile([1, 440], BF16, tag="ub", name="ub")
                nc.vector.tensor_copy(ub[:, :], ur[:, :])
                ua_bc = pc.tile([128, 400], BF16, tag="ua_bc", name="ua_bc")
                nc.gpsimd.partition_broadcast(ua_bc[:, :], ub[0:1, 40:440])
                uq_bc = pc.tile([128, 40], BF16, tag="uq_bc", name="uq_bc")
                nc.gpsimd.partition_broadcast(uq_bc[:, :], ub[0:1, 0:40])

                ua_v = ua_bc[:].rearrange("p (o l) -> p o l", o=1).broadcast_to((128, BS, 400))
                uq_v = uq_bc[:].rearrange("p (o m) -> p o m", o=1).broadcast_to((128, BS, 40))

                def tree_sum(eng, av, out, n):
                    # pairwise in-place halving keeps every op all-bf16
                    # (2x rate); a strided X-reduce would run at 1x.
                    while n > 25 and n % 2 == 0:
                        h = n // 2
                        eng.tensor_tensor(av[:, :, 0:h], av[:, :, 0:h],
                                          av[:, :, h:n], op=OP.add)
                        n = h
                    # free-axis reduce is DVE-only
                    nc.vector.reduce_sum(out, av[:, :, 0:n],
                                         axis=mybir.AxisListType.X, op=OP.add)

                scr = pc.tile([128, 400], BF16, tag="scr", name="scr")
                for t in range(NT):
                    av = a_sb[t][:].rearrange("p (b l) -> p b l", b=BS)
                    nc.vector.tensor_tensor(av, av, ua_v, op=OP.mult)
                    if t == 0:
                        # otherwise-idle scalar engine reduces one c-tile
                        for bb in range(BS):
                            nc.scalar.activation(
                                scr[:, :], av[:, bb, :], AF.Copy,
                                accum_out=ra_t[t][:, bb:bb + 1])
                    else:
                        tree_sum(nc.vector, av, ra_t[t][:, :], 400)
                for t in range(NT):
                    qv = q_sb[t][:].rearrange("p (b m) -> p b m", b=BS)
                    nc.vector.tensor_tensor(qv, qv, uq_v, op=OP.mult)
                    tree_sum(nc.vector, qv, rq_t[t][:, :], 40)

                # cosine similarity: reduce over c = 4 tiles x 128 partitions
                # via accumulating PE transposes: psum [BS,128] = sum_t P_t^T,
                # then a free-dim reduce gives the per-b column.
                def psum_all(tiles, tag):
                    tps = cps.tile([BS, 128], F32, tag="cts", name=f"{tag}tp")
                    for t in range(NT):
                        nc.tensor.matmul(tps[:, :], tiles[t][:, :], id_sb[:, :],
                                         is_transpose=True,
                                         start=(t == 0), stop=(t == NT - 1))
                    col = pc.tile([32, 1], F32, tag=f"{tag}c", name=f"{tag}c")
                    nc.vector.reduce_sum(col[:, :], tps[:, :],
                                         axis=mybir.AxisListType.X, op=OP.add)
                    return col

                pr = [pc.tile([128, BS], F32, tag=f"pr{t}", name=f"pr{t}") for t in range(NT)]
                pq = [pc.tile([128, BS], F32, tag=f"pq{t}", name=f"pq{t}") for t in range(NT)]
                pa = [pc.tile([128, BS], F32, tag=f"pa{t}", name=f"pa{t}") for t in range(NT)]
                for t in range(NT):
                    nc.vector.tensor_mul(pr[t][:, :], rq_t[t][:, :], ra_t[t][:, :])
                    nc.vector.tensor_mul(pq[t][:, :], rq_t[t][:, :], rq_t[t][:, :])
                    nc.vector.tensor_mul(pa[t][:, :], ra_t[t][:, :], ra_t[t][:, :])
                dot = psum_all(pr, "dt")
                qq = psum_all(pq, "qq")
                aa = psum_all(pa, "aa")

                nq = pc.tile([32, 1], F32, tag="nq", name="nq")
                na = pc.tile([32, 1], F32, tag="na", name="na")
                nc.scalar.activation(nq[:, :], qq[:, :], AF.Sqrt)
                nc.scalar.activation(na[:, :], aa[:, :], AF.Sqrt)
                nc.vector.tensor_scalar_max(nq[:, :], nq[:, :], 1e-6)
                nc.vector.tensor_scalar_max(na[:, :], na[:, :], 1e-6)
                den = pc.tile([32, 1], F32, tag="den", name="den")
                nc.vector.tensor_mul(den[:, :], nq[:, :], na[:, :])
                rden = pc.tile([32, 1], F32, tag="rden", name="rden")
                nc.vector.reciprocal(rden[:, :], den[:, :])
                res = pc.tile([32, 1], F32, tag="res", name="res")
                nc.vector.tensor_mul(res[:, :], dot[:, :], rden[:, :])
                nc.gpsimd.dma_start(out_d[:].rearrange("(a b) -> a b", b=1),
                                    res[:, :])

    nc.finalize()
    return nc


def _prep(question, answer, Wq, bq, Wa, ba, U):
    bf = ml_dtypes.bfloat16
    qs = question.reshape(N_CORES, BS, M, E)
    as_ = answer.reshape(N_CORES, BS, L, E)

    def enc_z8(x, T):
        # x: [BS, T, E] -> Z^T rows [BS, 8, 128, T] bf16 (ctx shifts baked in)
        xt = x.transpose(0, 2, 1)  # [BS, E, T]
        xtp = np.zeros((x.shape[0], E, T + 2), np.float32)
        xtp[:, :, 1:T + 1] = xt
        z = np.zeros((x.shape[0], 1024, T), dtype=bf)
        for i in range(3):
            z[:, i * E:(i + 1) * E, :] = xtp[:, :, i:i + T].astype(bf)
        return z.reshape(x.shape[0], 8, 128, T)

    def enc_xq8(x):
        # [BS, M, E] -> [8, 128, BS*M] bf16
        z = enc_z8(x, M)  # [BS, 8, 128, 40]
        return np.ascontiguousarray(z.transpose(1, 2, 0, 3)).reshape(8, 128, MT)

    def enc_w8(W):
        # W [C, 900] -> W^T padded [8, 128, CP] bf16
        o = np.zeros((1024, CP), dtype=bf)
        o[0:900, 0:C] = W.T.astype(bf)
        return o.reshape(8, 128, CP)

    up = np.zeros((CP, CP), dtype=bf)
    up[0:C, 0:C] = U.astype(bf)
    up = up.reshape(NT, 128, CP)

    def enc_b(b):
        o = np.zeros((CP,), np.float32)
        o[0:C] = b
        return np.ascontiguousarray(o.reshape(NT, 128).T)

    com = {
        "wqt": enc_w8(Wq), "wat": enc_w8(Wa), "ut": up,
        "bq": enc_b(bq), "ba": enc_b(ba),
        "ident": np.eye(128, dtype=np.float32),
        "ones": np.ones((1, 128), dtype=bf),
        "ones_col": np.ones((128, 1), dtype=bf),
    }
    maps = []
    for i in range(N_CORES):
        m = dict(com)
        m["xq"] = enc_xq8(qs[i])
        m["xa"] = enc_z8(as_[i], L)
        maps.append(m)
    return maps


def kernel(question, answer, Wq, bq, Wa, ba, U, _trace=False):
    if "nc" not in _CACHE:
        _CACHE["nc"] = _build()
    nc = _CACHE["nc"]
    maps = _prep(np.asarray(question), np.asarray(answer), np.asarray(Wq),
                 np.asarray(bq), np.asarray(Wa), np.asarray(ba), np.asarray(U))
    r = run_bass_kernel_spmd(nc, maps, list(range(N_CORES)), trace=_trace)
    _CACHE["last"] = r
    return np.concatenate([r.results[i]["out"] for i in range(N_CORES)])
